# revision 1
# baseline (speedup 1.0000x reference)
"""Bass/Trainium2 kernel for nn_CopyGenerator (8-core SPMD).

Sharding: 4-way vocab (tensor parallel) x 2-way rows (data parallel).
Core c = 4*h + q owns rows [2048h, 2048h+2048) and vocab columns
[8000q, 8000q+8000).  The softmax denominator needs a cross-vocab-shard
sum: one AllReduce over 4 ranks per tapered group of row-blocks (GS),
in two independent replica groups ([[0,1,2,3],[4,5,6,7]]) that pipeline
behind compute (~7-15us each vs ~30us for an 8-rank ring).  The copy
branch stays batch-sharded 8 ways (4 batches/core).  A tiny warmup NEFF
with one AllReduce runs first: the first collective after device boot
pays ~60-75us of channel start latency that would otherwise stall the
main kernel.

Per 128-row block:
  - PE: logits into PSUM (4 K-chunks x <=512-col matmuls, bf16),
    plus one N=1 matmul for the copy gate on the same stationaries.
  - ACT: Exp with accum_out (free row partial sums), exp values kept in
    SBUF fp16.  Sigmoid is computed via the Exp table too
    (sigmoid(x) = 1/(1+exp(-x))) so the ACT LUT never swaps.
  - After the group all-reduce: DVE scales exp by (1-gate)/S and the
    result is stored as bf16 (host upcasts; probs are ~1e-4 so bf16
    rounding is ~1e-7 absolute).
  - PAD masking: host zeroes W[PAD,:] (dead data in the reference), the
    resulting constant exp(0)=1 is subtracted from the reduced sum, and
    the host zeroes output column PAD.
  - Copy branch: fp16 matmul (one-hot src_map is exact in fp16) with an
    fp32-grade gate from a bf16 hi/lo split dot product.
"""

import os
import sys

for _p in ("/opt/trn_rl_repo", "/root/.axon_site/_ro/trn_rl_repo"):
    if os.path.isdir(_p) and _p not in sys.path:
        sys.path.insert(0, _p)

import numpy as np
import ml_dtypes

import concourse.bacc as bacc
import concourse.tile as tile
from concourse import mybir
from concourse.bass_utils import run_bass_kernel_spmd

# ---------------------------------------------------------------------------
# Problem dimensions (hardcoded per spec)
# ---------------------------------------------------------------------------
B, T, S, V, CV, D = 32, 128, 400, 32000, 600, 512
PAD = 1
NCORES = 8
NQ = 4                    # vocab shards
NH = 2                    # row halves
R = B * T                 # 4096 rows
VS = V // NQ              # 8000 vocab columns per core
RH = R // NH              # 2048 rows per core
RB = 128                  # rows per block (= one batch: T == 128)
NBL = RH // RB            # 16 row blocks per core
# tapered all-reduce groups: small first group fills the pipeline before the
# exp pool saturates; tiny last groups shrink the drain tail
GS = [2, 3, 3, 3, 3, 2]   # sums to NBL
NG = len(GS)
GOFF = [sum(GS[:i]) for i in range(NG)]
GRPOF = []                # block -> (group, index-in-group)
for _g, _n in enumerate(GS):
    for _j in range(_n):
        GRPOF.append((_g, _j))
LB = B // NCORES          # 4 local batches per core (copy branch)
KC = D // 128             # 4 contraction chunks
# vocab chunking within a block (PSUM: [128,1024]f32 = 2 banks)
VCH = [1024] * 7 + [832]  # = 8000
VOFF = [1024 * i for i in range(8)]
NVC = len(VCH)
# s-dim chunks for the copy branch: 400 = 128+128+128+16
SCH = [128, 128, 128, 16]
SOFF = [0, 128, 256, 384]

F32 = mybir.dt.float32
F16 = mybir.dt.float16
BF16 = mybir.dt.bfloat16

EXP_BUFS = 42   # in-flight exp tiles ([128,1024] f16)
OUT_BUFS = 2    # [128, 4096] bf16 output staging tiles (2 per block)


def _mm_splits(n):
    """Split a free-dim span into <=512 pieces aligned to 512 (PSUM banks)."""
    out = []
    off = 0
    while off < n:
        w = min(512, n - off)
        out.append((off, w))
        off += w
    return out


def build_program(with_bias: bool, b_copy: float, pad_corr: float):
    # Bacc (not plain Bass): its finalize() runs move_matmul_waits_to_ldweights
    # + generate_event_semaphores, which split multi-sem waits down to the
    # TRN2 limit of one wait per instruction — walrus rejects the IR otherwise.
    nc = bacc.Bacc()

    hT = nc.dram_tensor("hT", [D, RH], BF16, kind="ExternalInput")
    wT = nc.dram_tensor("wT", [D, VS], BF16, kind="ExternalInput")
    hTlh = nc.dram_tensor("hTlh", [D, LB * RB], BF16, kind="ExternalInput")
    hTll = nc.dram_tensor("hTll", [D, LB * RB], BF16, kind="ExternalInput")
    wch = nc.dram_tensor("wch", [D, 1], BF16, kind="ExternalInput")
    wcl = nc.dram_tensor("wcl", [D, 1], BF16, kind="ExternalInput")
    attnT = nc.dram_tensor("attnT", [S, LB * RB], F16, kind="ExternalInput")
    smap = nc.dram_tensor("smap", [LB, S, CV], F16, kind="ExternalInput")
    if with_bias:
        ebb = nc.dram_tensor("ebb", [128, VS], F32, kind="ExternalInput")

    og = nc.dram_tensor("og", [RH, VS], BF16, kind="ExternalOutput")
    oc = nc.dram_tensor("oc", [LB * RB, CV], F32, kind="ExternalOutput")

    with tile.TileContext(nc) as tc:
        with (
            tc.tile_pool(name="const", bufs=1) as const,
            tc.tile_pool(name="pm", bufs=2, space="PSUM") as pm,
            tc.tile_pool(name="pg", bufs=2, space="PSUM") as pg,
            tc.tile_pool(name="pc", bufs=1, space="PSUM") as pc,
            tc.tile_pool(name="expp", bufs=EXP_BUFS) as expp,
            tc.tile_pool(name="outp", bufs=OUT_BUFS) as outp,
            tc.tile_pool(name="ocp", bufs=2) as ocp,
            tc.tile_pool(name="smapp", bufs=4) as smapp,
            tc.tile_pool(name="small", bufs=10) as small,
            tc.tile_pool(name="gatep", bufs=NBL + LB) as gatep,
            tc.tile_pool(name="dram", bufs=1, space="DRAM") as dram,
        ):
            # ---------------- prologue: resident loads ----------------
            hT_t = []
            wT_t = []
            hTl_t = []
            wc_t = []
            attnT_t = []
            ebb_t = []
            # small copy-branch inputs on the gpsimd ring; the copy branch is
            # emitted before the main loop so PE has work during weight loads
            for k in range(KC):
                th = const.tile([128, LB * RB], BF16, tag=f"hTlh{k}", name=f"hTlh{k}")
                nc.sync.dma_start(th[:], hTlh[k * 128:(k + 1) * 128, :])
                tl = const.tile([128, LB * RB], BF16, tag=f"hTll{k}", name=f"hTll{k}")
                nc.sync.dma_start(tl[:], hTll[k * 128:(k + 1) * 128, :])
                hTl_t.append((th, tl))
                th = const.tile([128, 1], BF16, tag=f"wch{k}", name=f"wch{k}")
                nc.sync.dma_start(th[:], wch[k * 128:(k + 1) * 128, :])
                tl = const.tile([128, 1], BF16, tag=f"wcl{k}", name=f"wcl{k}")
                nc.sync.dma_start(tl[:], wcl[k * 128:(k + 1) * 128, :])
                wc_t.append((th, tl))
            for k in range(4):
                sk = SCH[k]
                t = const.tile([128, LB * RB], F16, tag=f"attnT{k}", name=f"attnT{k}")
                nc.sync.dma_start(t[:sk, :], attnT[SOFF[k]:SOFF[k] + sk, :])
                attnT_t.append(t)
            # big resident weights: hT on the SP ring, wT on the ACT ring
            # (two HWDGE rings run concurrently), split so early blocks'
            # dependencies land first
            for k in range(KC):
                t = const.tile([128, RH], BF16, tag=f"hT{k}", name=f"hT{k}")
                hT_t.append(t)
                t = const.tile([128, VS], BF16, tag=f"wT{k}", name=f"wT{k}")
                wT_t.append(t)
            for k in range(KC):
                nc.gpsimd.dma_start(hT_t[k][:], hT[k * 128:(k + 1) * 128, :])
            # wT spread across rings: k0/k1 on ACT ring, k2 on SP ring,
            # k3 on the gpsimd ring (after hT) — all three run concurrently
            w_slices = [(0, 1024), (1024, 2048), (2048, 4096), (4096, VS)]
            w_eng = [nc.scalar, nc.scalar, nc.scalar, nc.scalar]
            for (vo, ve) in w_slices:
                for k in range(KC):
                    w_eng[k].dma_start(
                        wT_t[k][:, vo:ve],
                        wT[k * 128:(k + 1) * 128, vo:ve],
                    )
            if with_bias:
                for i in range(NVC):
                    t = const.tile([128, VCH[i]], F32, tag=f"ebb{i}", name=f"ebb{i}")
                    nc.sync.dma_start(t[:], ebb[:, VOFF[i]:VOFF[i] + VCH[i]])
                    ebb_t.append(t)

            # ---------------- main loop ----------------
            exp_tiles = [[None] * NVC for _ in range(NBL)]
            e_tiles = [None] * NBL    # exp(-gate logit) per block [128,1] f32
            u_tiles = [None] * NBL    # 1 + e per block [128,1] f32
            sg_tiles = [None] * NG    # group local sums [128, GROUP]
            cc_out = [None] * NG      # group all-reduced sums (SBUF)

            def compute_gate(jb):
                cb = slice(jb * RB, (jb + 1) * RB)
                # gate matmul (bf16) -> psum [128, 1]
                gps = pg.tile([128, 1], F32, tag="gate", name="gate")
                for k in range(KC):
                    nc.tensor.matmul(
                        gps[:], hT_t[k][:, cb], wc_t[k][0][:],
                        start=(k == 0), stop=(k == KC - 1),
                    )
                # sigmoid via the Exp table: e = exp(-(x+b_copy))
                e = gatep.tile([128, 1], F32, tag="e", name="e")
                nc.scalar.activation(
                    e[:], gps[:], mybir.ActivationFunctionType.Exp,
                    bias=-float(b_copy), scale=-1.0,
                )
                u = gatep.tile([128, 1], F32, tag="u", name="u")
                nc.vector.tensor_scalar_add(u[:], e[:], 1.0)
                e_tiles[jb] = e
                u_tiles[jb] = u

            def compute_block(jb):
                cb = slice(jb * RB, (jb + 1) * RB)
                sp = small.tile([128, NVC], F32, tag="sp", name="sp")
                for i in range(NVC):
                    n = VCH[i]
                    ps = pm.tile([128, 1024], F32, tag="pm", name="pm")
                    for k in range(KC):
                        for (o, w) in _mm_splits(n):
                            nc.tensor.matmul(
                                ps[:, o:o + w],
                                hT_t[k][:, cb],
                                wT_t[k][:, VOFF[i] + o:VOFF[i] + o + w],
                                start=(k == 0), stop=(k == KC - 1),
                            )
                    ex = expp.tile([128, 1024], F16, tag="exp", name="exp")
                    if not with_bias:
                        nc.scalar.activation(
                            ex[:, :n], ps[:, :n],
                            mybir.ActivationFunctionType.Exp,
                            accum_out=sp[:, i:i + 1],
                        )
                    else:
                        nc.scalar.activation(
                            ex[:, :n], ps[:, :n],
                            mybir.ActivationFunctionType.Exp,
                        )
                        nc.vector.tensor_tensor(
                            ex[:, :n], ex[:, :n], ebb_t[i][:, :n],
                            mybir.AluOpType.mult,
                        )
                        nc.vector.reduce_sum(
                            sp[:, i:i + 1], ex[:, :n],
                            axis=mybir.AxisListType.X,
                        )
                    exp_tiles[jb][i] = ex
                g, j = GRPOF[jb]
                nc.vector.reduce_sum(
                    sg_tiles[g][:, j:j + 1], sp[:], axis=mybir.AxisListType.X
                )

            def scale_block(jb):
                g, j = GRPOF[jb]
                sgl = cc_out[g]
                # m = (1-gate)/S = e / ((1+e) * (S_allreduce - pad_corr))
                corr = small.tile([128, 1], F32, tag="corr", name="corr")
                nc.vector.tensor_scalar_add(corr[:], sgl[:, j:j + 1], -pad_corr)
                v = small.tile([128, 1], F32, tag="v", name="v")
                nc.vector.tensor_scalar(
                    v[:], corr[:], u_tiles[jb][:], None, mybir.AluOpType.mult
                )
                rec = small.tile([128, 1], F32, tag="rec", name="rec")
                nc.vector.reciprocal(rec[:], v[:])
                m = small.tile([128, 1], F32, tag="m", name="m")
                nc.vector.tensor_scalar(
                    m[:], rec[:], e_tiles[jb][:], None, mybir.AluOpType.mult
                )
                # scale exp chunks into bf16 staging tiles, 2 stores per block
                for half in range(2):
                    hn = 4096 if half == 0 else VS - 4096
                    ot = outp.tile([128, 4096], BF16, tag="ot", name="ot")
                    for i in range(4 * half, 4 * half + 4):
                        n = VCH[i]
                        oo = VOFF[i] - 4096 * half
                        nc.vector.tensor_scalar(
                            ot[:, oo:oo + n],
                            exp_tiles[jb][i][:, :n], m[:], None,
                            mybir.AluOpType.mult,
                        )
                        exp_tiles[jb][i] = None
                    nc.sync.dma_start(
                        og[jb * RB:(jb + 1) * RB, 4096 * half:4096 * half + hn],
                        ot[:, :hn],
                    )

            # ---------------- copy branch (batch-sharded) ----------------
            def emit_copy_branch():
                # emitted first: no dependence on collectives or the big weights
                for l in range(LB):
                    tb = slice(l * RB, (l + 1) * RB)
                    # local gate: fp32-grade dot via bf16 hi/lo split
                    gps = pg.tile([128, 1], F32, tag="gate", name="gate")
                    nmm = 3 * KC
                    imm = 0
                    for k in range(KC):
                        for (a, b_) in ((0, 0), (0, 1), (1, 0)):
                            nc.tensor.matmul(
                                gps[:], hTl_t[k][a][:, tb], wc_t[k][b_][:],
                                start=(imm == 0), stop=(imm == nmm - 1),
                            )
                            imm += 1
                    el = gatep.tile([128, 1], F32, tag="el", name="el")
                    nc.scalar.activation(
                        el[:], gps[:], mybir.ActivationFunctionType.Exp,
                        bias=-float(b_copy), scale=-1.0,
                    )
                    ul = gatep.tile([128, 1], F32, tag="ul", name="ul")
                    nc.vector.tensor_scalar_add(ul[:], el[:], 1.0)
                    gl = gatep.tile([128, 1], F32, tag="gl", name="gl")
                    nc.vector.reciprocal(gl[:], ul[:])
                    cps = pc.tile([128, CV], F32, tag="cp", name="cp")
                    for k in range(4):
                        sk = SCH[k]
                        sm = smapp.tile([128, CV], F16, tag="sm", name="sm")
                        nc.scalar.dma_start(
                            sm[:sk, :], smap[l, SOFF[k]:SOFF[k] + sk, :]
                        )
                        for (o, w) in _mm_splits(CV):
                            nc.tensor.matmul(
                                cps[:, o:o + w],
                                attnT_t[k][:sk, tb],
                                sm[:sk, o:o + w],
                                start=(k == 0), stop=(k == 3),
                            )
                    oct_ = ocp.tile([128, CV], F32, tag="oct", name="oct")
                    nc.vector.tensor_scalar(
                        oct_[:], cps[:], gl[:], None, mybir.AluOpType.mult
                    )
                    nc.sync.dma_start(oc[tb, :], oct_[:])


            emit_copy_branch()
            # all 16 gates up-front: they only need hT + w_copy (small, fast
            # loads), so they fill the PE while the 8 MB wT shard streams in
            for jb in range(NBL):
                compute_gate(jb)

            for g in range(NG):
                gn = GS[g]
                sg_tiles[g] = small.tile([128, gn], F32, tag="sg", name="sg")
                for j in range(gn):
                    compute_block(GOFF[g] + j)
                # all-reduce this group's local sums across the 4 vocab shards
                cin = dram.tile([128, gn], F32, tag=f"cin{g}", name=f"cin{g}")
                cout = dram.tile([128, gn], F32, tag=f"cout{g}", name=f"cout{g}")
                nc.gpsimd.dma_start(cin[:], sg_tiles[g][:])
                nc.gpsimd.collective_compute(
                    "AllReduce",
                    mybir.AluOpType.add,
                    replica_groups=[[0, 1, 2, 3], [4, 5, 6, 7]],
                    ins=[cin.opt()],
                    outs=[cout.opt()],
                )
                sgl = small.tile([128, gn], F32, tag="sgl", name="sgl")
                nc.gpsimd.dma_start(sgl[:], cout[:])
                cc_out[g] = sgl
                for j in range(gn):
                    scale_block(GOFF[g] + j)


    nc.finalize()
    return nc


_warmed_up = False


def _warmup_collectives():
    """Run a minimal NEFF with one AllReduce so the collective channel
    (ncfw firmware / TOPSP) is warm before the main kernel executes —
    the first collective after boot costs ~60-75us of start latency."""
    global _warmed_up
    if _warmed_up:
        return
    nc = bacc.Bacc()
    x = nc.dram_tensor("x", [128, 4], F32, kind="ExternalInput")
    y = nc.dram_tensor("y", [128, 4], F32, kind="ExternalOutput")
    with tile.TileContext(nc) as tc:
        with (
            tc.tile_pool(name="sb", bufs=2) as sb,
            tc.tile_pool(name="dr", bufs=2, space="DRAM") as dr,
        ):
            t = sb.tile([128, 4], F32, tag="t", name="t")
            nc.sync.dma_start(t[:], x[:])
            bi = dr.tile([128, 4], F32, tag="bi", name="bi")
            bo = dr.tile([128, 4], F32, tag="bo", name="bo")
            nc.sync.dma_start(bi[:], t[:])
            nc.gpsimd.collective_compute(
                "AllReduce",
                mybir.AluOpType.add,
                replica_groups=[[0, 1, 2, 3], [4, 5, 6, 7]],
                ins=[bi.opt()],
                outs=[bo.opt()],
            )
            t2 = sb.tile([128, 4], F32, tag="t2", name="t2")
            nc.sync.dma_start(t2[:], bo[:])
            nc.sync.dma_start(y[:], t2[:])
    nc.finalize()
    z = np.zeros((128, 4), np.float32)
    run_bass_kernel_spmd(nc, [{"x": z}] * NCORES, core_ids=list(range(NCORES)))
    _warmed_up = True


def kernel(hidden, copy_attn, src_map, W, b, w_copy, b_copy, _trace=False):
    hidden = np.asarray(hidden, np.float32)
    copy_attn = np.asarray(copy_attn, np.float32)
    src_map = np.asarray(src_map, np.float32)
    W = np.asarray(W, np.float32)
    b = np.asarray(b, np.float32)
    w_copy = np.asarray(w_copy, np.float32)
    b_copy_f = float(np.asarray(b_copy))

    with_bias = bool(np.any(b != 0.0))
    pad_corr = float(np.exp(b[PAD])) if with_bias else 1.0

    # host-side shard prep (layout only; W[PAD,:] is dead data in the ref)
    Wz = W.copy()
    Wz[PAD, :] = 0.0
    WT = np.ascontiguousarray(Wz.T).astype(ml_dtypes.bfloat16)      # [D, V]
    hT_f = np.ascontiguousarray(hidden.T)                            # [D, R] f32
    hT_b = hT_f.astype(ml_dtypes.bfloat16)
    hT_lo = (hT_f - hT_b.astype(np.float32)).astype(ml_dtypes.bfloat16)
    wc32 = w_copy.reshape(D, 1).astype(np.float32)
    wc_hi = wc32.astype(ml_dtypes.bfloat16)
    wc_lo = (wc32 - wc_hi.astype(np.float32)).astype(ml_dtypes.bfloat16)
    attnT_full = np.ascontiguousarray(copy_attn.T).astype(np.float16)  # [S, R]
    smap16 = src_map.astype(np.float16)                              # [B,S,CV]

    _warmup_collectives()
    nc = build_program(with_bias, b_copy_f, pad_corr)

    in_maps = []
    for c in range(NCORES):
        h, q = divmod(c, NQ)
        rows = slice(h * RH, (h + 1) * RH)
        crows = slice(c * LB * RB, (c + 1) * LB * RB)
        m = {
            "hT": np.ascontiguousarray(hT_b[:, rows]),
            "wT": np.ascontiguousarray(WT[:, q * VS:(q + 1) * VS]),
            "hTlh": np.ascontiguousarray(hT_b[:, crows]),
            "hTll": np.ascontiguousarray(hT_lo[:, crows]),
            "wch": wc_hi,
            "wcl": wc_lo,
            "attnT": np.ascontiguousarray(attnT_full[:, crows]),
            "smap": np.ascontiguousarray(smap16[c * LB:(c + 1) * LB]),
        }
        if with_bias:
            eb = np.exp(b[q * VS:(q + 1) * VS].astype(np.float64)).astype(
                np.float32
            )
            m["ebb"] = np.ascontiguousarray(
                np.broadcast_to(eb[None, :], (128, VS))
            )
        in_maps.append(m)

    trace_cores = None
    if os.environ.get("TRACE_ALL_CORES"):
        trace_cores = list(range(NCORES))
    res = run_bass_kernel_spmd(
        nc, in_maps, core_ids=list(range(NCORES)), trace=_trace,
        trace_cores=trace_cores,
    )

    out = np.empty((R, V + CV), np.float32)
    for c in range(NCORES):
        h, q = divmod(c, NQ)
        out[h * RH:(h + 1) * RH, q * VS:(q + 1) * VS] = (
            res.results[c]["og"].astype(np.float32)
        )
        out[c * LB * RB:(c + 1) * LB * RB, V:] = res.results[c]["oc"]
    out[:, PAD] = 0.0

    if _trace:
        kernel.last_results = res
    return out


kernel.last_results = None



# revision 9
# speedup vs baseline: 1.2782x; 1.2782x over previous
"""Bass/Trainium2 kernel for nn_CopyGenerator (8-core SPMD), v2 (fp8).

Sharding: 4-way vocab (tensor parallel) x 2-way rows (data parallel).
Core c = 4*h + q owns rows [2048h, 2048h+2048) and vocab columns
[8000q, 8000q+8000).  The cross-vocab-shard softmax sum is one AllReduce
over 4 ranks per tapered group of row-blocks (GS), in two independent
replica groups ([[0,1,2,3],[4,5,6,7]]).  The copy branch stays
batch-sharded 8 ways (4 batches/core).  A tiny warmup NEFF with one
AllReduce runs first (collective channel start latency ~60-75us).

v2 changes vs the bf16 baseline (374us):
  - Logits matmul in fp8e4 DoubleRow (2 K-planes per PE pass): W is
    pre-scaled by 32 on the host so its ~N(0,0.02) entries stay in
    fp8e4 normal range; the exp activation applies scale=1/32.
    Halves PE time (272us busy -> ~140us).
  - exp tiles stored fp8 (values in [e^-3, e^3], 3% rel err vs a 2e-2
    budget) - 2KB/partition/tile lets the pipeline run 48 tiles deep
    across the AllReduce latency.
  - One exp per 2048-col PSUM tile (PSUM = 2 x [128,2048] covering all
    8 banks; gates and the copy branch use slices of the same pool)
    instead of per 1024 - halves ACT instruction overhead.
  - Output og stored as fp8 of 8192*prob (host divides by 8192 on
    upcast) - halves the dominant og store traffic vs bf16.
  - The per-group sum reduce moved off the DVE FIFO (to gpsimd) and the
    AllReduce result fetch to the SP queue, so AR g+1's posting no
    longer chains behind AR g's scale drain on DVE.  In the baseline
    that false dependency serialized the 6 ARs at ~40us each across the
    whole back half of the kernel.
  - Gate logit matmuls in plain fp8 (N=1); e8 = 8192*exp(-x) comes
    straight out of the Exp activation via bias=ln(8192)-b_copy.
"""

import math
import os
import sys

for _p in ("/opt/trn_rl_repo", "/root/.axon_site/_ro/trn_rl_repo"):
    if os.path.isdir(_p) and _p not in sys.path:
        sys.path.insert(0, _p)

import numpy as np
import ml_dtypes

import concourse.bacc as bacc
import concourse.tile as tile
from concourse import mybir
from concourse.bass_utils import run_bass_kernel_spmd

# ---------------------------------------------------------------------------
# Problem dimensions (hardcoded per spec)
# ---------------------------------------------------------------------------
B, T, S, V, CV, D = 32, 128, 400, 32000, 600, 512
PAD = 1
NCORES = 8
NQ = 4                    # vocab shards
NH = 2                    # row halves
R = B * T                 # 4096 rows
VS = V // NQ              # 8000 vocab columns per core
RH = R // NH              # 2048 rows per core
RB = 128                  # rows per block (= one batch: T == 128)
NBL = RH // RB            # 16 row blocks per core
GS = [3, 3, 3, 3, 3, 1]   # all-reduce groups; tiny last group = short tail
NG = len(GS)
GOFF = [sum(GS[:i]) for i in range(NG)]
GRPOF = []                # block -> (group, index-in-group)
for _g, _n in enumerate(GS):
    for _j in range(_n):
        GRPOF.append((_g, _j))
LB = B // NCORES          # 4 local batches per core (copy branch)
KC = D // 128             # 4 contraction chunks of 128
# vocab chunking within a block (PSUM: [128,2048]f32 = 4 banks)
VCH = [2048, 2048, 2048, 1856]   # = 8000
VOFF = [0, 2048, 4096, 6144]
NVC = len(VCH)
# s-dim chunks for the copy branch: 400 = 128+128+128+16
SCH = [128, 128, 128, 16]
SOFF = [0, 128, 256, 384]

WSCALE = 32.0             # host multiplies W/w_copy by this before fp8 cast
OUT_SCALE = 8192.0        # og holds OUT_SCALE * prob in fp8

F32 = mybir.dt.float32
F16 = mybir.dt.float16
BF16 = mybir.dt.bfloat16
FP8 = mybir.dt.float8e4
NP_FP8 = ml_dtypes.float8_e4m3

EXP_BUFS = 48   # in-flight exp tiles ([128,2048] fp8)
DR = mybir.MatmulPerfMode.DoubleRow


def _mm_splits(n):
    """Split a free-dim span into <=512 pieces aligned to 512 (PSUM banks)."""
    out = []
    off = 0
    while off < n:
        w = min(512, n - off)
        out.append((off, w))
        off += w
    return out


def build_program(with_bias: bool, b_copy: float, pad_corr: float):
    nc = bacc.Bacc()

    hT8 = nc.dram_tensor("hT8", [128, KC, RH], FP8, kind="ExternalInput")
    wT8 = nc.dram_tensor("wT8", [128, KC, VS], FP8, kind="ExternalInput")
    wc8 = nc.dram_tensor("wc8", [128, KC], FP8, kind="ExternalInput")
    hTlh = nc.dram_tensor("hTlh", [D, LB * RB], BF16, kind="ExternalInput")
    hTll = nc.dram_tensor("hTll", [D, LB * RB], BF16, kind="ExternalInput")
    wch = nc.dram_tensor("wch", [D, 1], BF16, kind="ExternalInput")
    wcl = nc.dram_tensor("wcl", [D, 1], BF16, kind="ExternalInput")
    attnT = nc.dram_tensor("attnT", [S, LB * RB], F16, kind="ExternalInput")
    smap = nc.dram_tensor("smap", [LB, S, CV], F16, kind="ExternalInput")
    if with_bias:
        ebb = nc.dram_tensor("ebb", [128, VS], F32, kind="ExternalInput")

    og = nc.dram_tensor("og", [RH, VS], FP8, kind="ExternalOutput")
    oc = nc.dram_tensor("oc", [LB * RB, CV], F32, kind="ExternalOutput")

    with tile.TileContext(nc) as tc:
        with (
            tc.tile_pool(name="const", bufs=1) as const,
            tc.tile_pool(name="pm", bufs=2, space="PSUM") as pm,
            tc.tile_pool(name="expp", bufs=EXP_BUFS) as expp,
            tc.tile_pool(name="outp", bufs=2) as outp,
            tc.tile_pool(name="ocp", bufs=2) as ocp,
            tc.tile_pool(name="smapp", bufs=4) as smapp,
            tc.tile_pool(name="small", bufs=10) as small,
            tc.tile_pool(name="gatep", bufs=NBL) as gatep,
            tc.tile_pool(name="dram", bufs=1, space="DRAM") as dram,
        ):
            # ---------------- prologue: resident loads ----------------
            hTl_t = []
            wc_t = []
            attnT_t = []
            ebb_t = []
            # small copy-branch inputs on the gpsimd ring; the copy branch is
            # emitted before the main loop so PE has work during weight loads
            for k in range(KC):
                th = const.tile([128, LB * RB], BF16, tag=f"hTlh{k}", name=f"hTlh{k}")
                nc.gpsimd.dma_start(th[:], hTlh[k * 128:(k + 1) * 128, :])
                tl = const.tile([128, LB * RB], BF16, tag=f"hTll{k}", name=f"hTll{k}")
                nc.gpsimd.dma_start(tl[:], hTll[k * 128:(k + 1) * 128, :])
                hTl_t.append((th, tl))
                th = const.tile([128, 1], BF16, tag=f"wch{k}", name=f"wch{k}")
                nc.gpsimd.dma_start(th[:], wch[k * 128:(k + 1) * 128, :])
                tl = const.tile([128, 1], BF16, tag=f"wcl{k}", name=f"wcl{k}")
                nc.gpsimd.dma_start(tl[:], wcl[k * 128:(k + 1) * 128, :])
                wc_t.append((th, tl))
            for k in range(4):
                sk = SCH[k]
                t = const.tile([128, LB * RB], F16, tag=f"attnT{k}", name=f"attnT{k}")
                nc.gpsimd.dma_start(t[:sk, :], attnT[SOFF[k]:SOFF[k] + sk, :])
                attnT_t.append(t)
            # fp8 gen-branch residents: hidden + gate weight on gpsimd,
            # the 4MB W shard split per (vocab chunk, k-plane) across the
            # SP and ACT rings so block 0 chunk 0's dependencies land first
            hT8_t = const.tile([128, KC, RH], FP8, tag="hT8", name="hT8")
            nc.gpsimd.dma_start(hT8_t[:], hT8[:])
            wc8_t = const.tile([128, KC], FP8, tag="wc8", name="wc8")
            nc.gpsimd.dma_start(wc8_t[:], wc8[:])
            wT8_t = const.tile([128, KC, VS], FP8, tag="wT8", name="wT8")
            w_eng = [nc.sync, nc.scalar]
            ie = 0
            for c in range(NVC):
                vo, ve = VOFF[c], VOFF[c] + VCH[c]
                for k in range(KC):
                    w_eng[ie % 2].dma_start(
                        wT8_t[:, k:k + 1, vo:ve], wT8[:, k:k + 1, vo:ve]
                    )
                    ie += 1
            if with_bias:
                for i in range(NVC):
                    t = const.tile([128, VCH[i]], F32, tag=f"ebb{i}", name=f"ebb{i}")
                    nc.sync.dma_start(t[:], ebb[:, VOFF[i]:VOFF[i] + VCH[i]])
                    ebb_t.append(t)

            # gate-exp bias constants as [128,1] tiles (only 0.0/1.0 floats
            # are pre-registered in the const-AP database)
            be8_t = const.tile([128, 1], F32, tag="be8", name="be8")
            nc.gpsimd.memset(be8_t[:], math.log(OUT_SCALE) - float(b_copy))
            bcl_t = const.tile([128, 1], F32, tag="bcl", name="bcl")
            nc.gpsimd.memset(bcl_t[:], -float(b_copy))

            # ---------------- main loop state ----------------
            exp_tiles = [[None] * NVC for _ in range(NBL)]
            e8_tiles = [None] * NBL   # 8192*exp(-gate logit) per block
            u_tiles = [None] * NBL    # 1 + e per block [128,1] f32
            cin_t = [None] * NG       # group AR inputs [128, 4*gn] (DRAM)
            cc_out = [None] * NG      # group all-reduced sums (DRAM)

            def compute_gate(jb):
                cb = slice(jb * RB, (jb + 1) * RB)
                # gate matmul (plain fp8, N=1) -> psum [128, 1]
                gps = pm.tile([128, 2048], F32, tag="pm", name="gps")
                for k in range(KC):
                    nc.tensor.matmul(
                        gps[:, 0:1], hT8_t[:, k:k + 1, cb], wc8_t[:, k:k + 1],
                        start=(k == 0), stop=(k == KC - 1),
                    )
                # e8 = OUT_SCALE * exp(-(x/WSCALE + b_copy)) via the Exp table
                e8 = gatep.tile([128, 1], F32, tag="e8", name="e8")
                nc.scalar.activation(
                    e8[:], gps[:, 0:1], mybir.ActivationFunctionType.Exp,
                    bias=be8_t[:],
                    scale=-1.0 / WSCALE,
                )
                # u = 1 + e = 1 + e8/OUT_SCALE
                u = gatep.tile([128, 1], F32, tag="u", name="u")
                nc.vector.tensor_scalar(
                    u[:], e8[:], 1.0 / OUT_SCALE, 1.0,
                    mybir.AluOpType.mult, mybir.AluOpType.add,
                )
                e8_tiles[jb] = e8
                u_tiles[jb] = u

            def compute_block(jb):
                cb = slice(jb * RB, (jb + 1) * RB)
                sp = small.tile([128, NVC], F32, tag="sp", name="sp")
                for i in range(NVC):
                    n = VCH[i]
                    ps = pm.tile([128, 2048], F32, tag="pm", name="pm")
                    for j in range(2):  # kk pairs (DoubleRow: 2 K-planes)
                        for (o, w) in _mm_splits(n):
                            nc.tensor.matmul(
                                ps[:, o:o + w],
                                hT8_t[:, 2 * j:2 * j + 2, cb],
                                wT8_t[:, 2 * j:2 * j + 2,
                                      VOFF[i] + o:VOFF[i] + o + w],
                                start=(j == 0), stop=(j == 1),
                                perf_mode=DR,
                            )
                    ex = expp.tile([128, 2048], FP8, tag="exp", name="exp")
                    if not with_bias:
                        nc.scalar.activation(
                            ex[:, :n], ps[:, :n],
                            mybir.ActivationFunctionType.Exp,
                            scale=1.0 / WSCALE,
                            accum_out=sp[:, i:i + 1],
                        )
                    else:
                        nc.scalar.activation(
                            ex[:, :n], ps[:, :n],
                            mybir.ActivationFunctionType.Exp,
                            scale=1.0 / WSCALE,
                        )
                        nc.vector.tensor_tensor(
                            ex[:, :n], ex[:, :n], ebb_t[i][:, :n],
                            mybir.AluOpType.mult,
                        )
                        nc.vector.reduce_sum(
                            sp[:, i:i + 1], ex[:, :n],
                            axis=mybir.AxisListType.X,
                        )
                    exp_tiles[jb][i] = ex
                g, j = GRPOF[jb]
                # off the DVE FIFO: ship the raw per-chunk sums; the AR sums
                # elementwise and the 4-way fold moves to the scale side.
                # AR posting never chains behind the previous group's scales.
                nc.gpsimd.dma_start(cin_t[g][:, NVC * j:NVC * (j + 1)], sp[:])

            def scale_block(jb, sgl):
                g, j = GRPOF[jb]
                # m8 = OUT_SCALE*(1-gate)/S = e8 / ((1+e) * (S_ar - pad_corr))
                ssum = small.tile([128, 1], F32, tag="ssum", name="ssum")
                nc.vector.reduce_sum(
                    ssum[:], sgl[:, NVC * j:NVC * (j + 1)],
                    axis=mybir.AxisListType.X,
                )
                corr = small.tile([128, 1], F32, tag="corr", name="corr")
                nc.vector.tensor_scalar_add(corr[:], ssum[:], -pad_corr)
                v = small.tile([128, 1], F32, tag="v", name="v")
                nc.vector.tensor_scalar(
                    v[:], corr[:], u_tiles[jb][:], None, mybir.AluOpType.mult
                )
                rec = small.tile([128, 1], F32, tag="rec", name="rec")
                nc.vector.reciprocal(rec[:], v[:])
                m8 = small.tile([128, 1], F32, tag="m8", name="m8")
                nc.vector.tensor_scalar(
                    m8[:], rec[:], e8_tiles[jb][:], None, mybir.AluOpType.mult
                )
                ot = outp.tile([128, VS], FP8, tag="ot", name="ot")
                for i in range(NVC):
                    n = VCH[i]
                    nc.vector.tensor_scalar(
                        ot[:, VOFF[i]:VOFF[i] + n],
                        exp_tiles[jb][i][:, :n], m8[:], None,
                        mybir.AluOpType.mult,
                    )
                    exp_tiles[jb][i] = None
                nc.sync.dma_start(og[jb * RB:(jb + 1) * RB, :], ot[:])

            # ---------------- copy branch (batch-sharded) ----------------
            def emit_copy_branch():
                # emitted first: no dependence on collectives or the big weights
                for l in range(LB):
                    tb = slice(l * RB, (l + 1) * RB)
                    # local gate: fp32-grade dot via bf16 hi/lo split
                    gps = pm.tile([128, 2048], F32, tag="pm", name="cgps")
                    nmm = 3 * KC
                    imm = 0
                    for k in range(KC):
                        for (a, b_) in ((0, 0), (0, 1), (1, 0)):
                            nc.tensor.matmul(
                                gps[:, 0:1], hTl_t[k][a][:, tb], wc_t[k][b_][:],
                                start=(imm == 0), stop=(imm == nmm - 1),
                            )
                            imm += 1
                    el = gatep.tile([128, 1], F32, tag="el", name="el")
                    nc.scalar.activation(
                        el[:], gps[:, 0:1], mybir.ActivationFunctionType.Exp,
                        bias=bcl_t[:], scale=-1.0,
                    )
                    ul = gatep.tile([128, 1], F32, tag="ul", name="ul")
                    nc.vector.tensor_scalar_add(ul[:], el[:], 1.0)
                    gl = gatep.tile([128, 1], F32, tag="gl", name="gl")
                    nc.vector.reciprocal(gl[:], ul[:])
                    cps = pm.tile([128, 2048], F32, tag="pm", name="cps")
                    for k in range(4):
                        sk = SCH[k]
                        sm = smapp.tile([128, CV], F16, tag="sm", name="sm")
                        nc.scalar.dma_start(
                            sm[:sk, :], smap[l, SOFF[k]:SOFF[k] + sk, :]
                        )
                        for (o, w) in _mm_splits(CV):
                            nc.tensor.matmul(
                                cps[:, o:o + w],
                                attnT_t[k][:sk, tb],
                                sm[:sk, o:o + w],
                                start=(k == 0), stop=(k == 3),
                            )
                    oct_ = ocp.tile([128, CV], F32, tag="oct", name="oct")
                    nc.vector.tensor_scalar(
                        oct_[:], cps[:, :CV], gl[:], None, mybir.AluOpType.mult
                    )
                    nc.sync.dma_start(oc[tb, :], oct_[:])

            emit_copy_branch()
            # all 16 gates up-front: they only need hT8 + wc8 (small, fast
            # loads), so they fill the PE while the 4 MB wT8 shard streams in
            for jb in range(NBL):
                compute_gate(jb)

            for g in range(NG):
                gn = GS[g]
                cin_t[g] = dram.tile(
                    [128, NVC * gn], F32, tag=f"cin{g}", name=f"cin{g}"
                )
                for j in range(gn):
                    compute_block(GOFF[g] + j)
                # all-reduce this group's local sums across the 4 vocab shards
                cout = dram.tile(
                    [128, NVC * gn], F32, tag=f"cout{g}", name=f"cout{g}"
                )
                nc.gpsimd.collective_compute(
                    "AllReduce",
                    mybir.AluOpType.add,
                    replica_groups=[[0, 1, 2, 3], [4, 5, 6, 7]],
                    ins=[cin_t[g].opt()],
                    outs=[cout.opt()],
                )
                cc_out[g] = cout
                # result fetch on the SP queue (not gpsimd: later ARs must
                # keep posting; not DVE: the trigger's wait would stall it)
                sgl = small.tile([128, NVC * gn], F32, tag="sgl", name="sgl")
                nc.sync.dma_start(sgl[:], cout[:])
                for j in range(gn):
                    scale_block(GOFF[g] + j, sgl)

    nc.finalize()
    return nc


_warmed_up = False


def _warmup_collectives():
    """Run a minimal NEFF with one AllReduce so the collective channel
    (ncfw firmware / TOPSP) is warm before the main kernel executes —
    the first collective after boot costs ~60-75us of start latency."""
    global _warmed_up
    if _warmed_up:
        return
    nc = bacc.Bacc()
    x = nc.dram_tensor("x", [128, 4], F32, kind="ExternalInput")
    y = nc.dram_tensor("y", [128, 4], F32, kind="ExternalOutput")
    with tile.TileContext(nc) as tc:
        with (
            tc.tile_pool(name="sb", bufs=2) as sb,
            tc.tile_pool(name="dr", bufs=2, space="DRAM") as dr,
        ):
            t = sb.tile([128, 4], F32, tag="t", name="t")
            nc.sync.dma_start(t[:], x[:])
            bi = dr.tile([128, 4], F32, tag="bi", name="bi")
            bo = dr.tile([128, 4], F32, tag="bo", name="bo")
            nc.sync.dma_start(bi[:], t[:])
            nc.gpsimd.collective_compute(
                "AllReduce",
                mybir.AluOpType.add,
                replica_groups=[[0, 1, 2, 3], [4, 5, 6, 7]],
                ins=[bi.opt()],
                outs=[bo.opt()],
            )
            t2 = sb.tile([128, 4], F32, tag="t2", name="t2")
            nc.sync.dma_start(t2[:], bo[:])
            nc.sync.dma_start(y[:], t2[:])
    nc.finalize()
    z = np.zeros((128, 4), np.float32)
    run_bass_kernel_spmd(nc, [{"x": z}] * NCORES, core_ids=list(range(NCORES)))
    _warmed_up = True


def kernel(hidden, copy_attn, src_map, W, b, w_copy, b_copy, _trace=False):
    hidden = np.asarray(hidden, np.float32)
    copy_attn = np.asarray(copy_attn, np.float32)
    src_map = np.asarray(src_map, np.float32)
    W = np.asarray(W, np.float32)
    b = np.asarray(b, np.float32)
    w_copy = np.asarray(w_copy, np.float32)
    b_copy_f = float(np.asarray(b_copy))

    with_bias = bool(np.any(b != 0.0))
    pad_corr = float(np.exp(b[PAD])) if with_bias else 1.0

    # host-side shard prep (layout only; W[PAD,:] is dead data in the ref)
    Wz = W.copy()
    Wz[PAD, :] = 0.0
    # [128, KC, V]: fp8 of 32*W.T, partition-major for a single-tile resident
    W8 = np.ascontiguousarray(
        (Wz.T * WSCALE).reshape(KC, 128, V).transpose(1, 0, 2)
    ).astype(NP_FP8)
    hT_f = np.ascontiguousarray(hidden.T)                            # [D, R] f32
    h8 = np.ascontiguousarray(
        hT_f.reshape(KC, 128, R).transpose(1, 0, 2)
    ).astype(NP_FP8)                                                 # [128, KC, R]
    wc8_full = np.ascontiguousarray(
        (w_copy * WSCALE).reshape(KC, 128).T
    ).astype(NP_FP8)                                                 # [128, KC]
    hT_b = hT_f.astype(ml_dtypes.bfloat16)
    hT_lo = (hT_f - hT_b.astype(np.float32)).astype(ml_dtypes.bfloat16)
    wc32 = w_copy.reshape(D, 1).astype(np.float32)
    wc_hi = wc32.astype(ml_dtypes.bfloat16)
    wc_lo = (wc32 - wc_hi.astype(np.float32)).astype(ml_dtypes.bfloat16)
    attnT_full = np.ascontiguousarray(copy_attn.T).astype(np.float16)  # [S, R]
    smap16 = src_map.astype(np.float16)                              # [B,S,CV]

    _warmup_collectives()
    nc = build_program(with_bias, b_copy_f, pad_corr)

    in_maps = []
    for c in range(NCORES):
        h, q = divmod(c, NQ)
        rows = slice(h * RH, (h + 1) * RH)
        crows = slice(c * LB * RB, (c + 1) * LB * RB)
        m = {
            "hT8": np.ascontiguousarray(h8[:, :, rows]),
            "wT8": np.ascontiguousarray(W8[:, :, q * VS:(q + 1) * VS]),
            "wc8": wc8_full,
            "hTlh": np.ascontiguousarray(hT_b[:, crows]),
            "hTll": np.ascontiguousarray(hT_lo[:, crows]),
            "wch": wc_hi,
            "wcl": wc_lo,
            "attnT": np.ascontiguousarray(attnT_full[:, crows]),
            "smap": np.ascontiguousarray(smap16[c * LB:(c + 1) * LB]),
        }
        if with_bias:
            eb = np.exp(b[q * VS:(q + 1) * VS].astype(np.float64)).astype(
                np.float32
            )
            m["ebb"] = np.ascontiguousarray(
                np.broadcast_to(eb[None, :], (128, VS))
            )
        in_maps.append(m)

    trace_cores = None
    if os.environ.get("TRACE_ALL_CORES"):
        trace_cores = list(range(NCORES))
    res = run_bass_kernel_spmd(
        nc, in_maps, core_ids=list(range(NCORES)), trace=_trace,
        trace_cores=trace_cores,
    )

    out = np.empty((R, V + CV), np.float32)
    inv = np.float32(1.0 / OUT_SCALE)
    for c in range(NCORES):
        h, q = divmod(c, NQ)
        out[h * RH:(h + 1) * RH, q * VS:(q + 1) * VS] = (
            res.results[c]["og"].astype(np.float32) * inv
        )
        out[c * LB * RB:(c + 1) * LB * RB, V:] = res.results[c]["oc"]
    out[:, PAD] = 0.0

    if _trace:
        kernel.last_results = res
    return out


kernel.last_results = None


# revision 16
# speedup vs baseline: 1.4603x; 1.1425x over previous
"""Bass/Trainium2 kernel for nn_CopyGenerator (8-core SPMD), v2 (fp8).

Sharding: 4-way vocab (tensor parallel) x 2-way rows (data parallel).
Core c = 4*h + q owns rows [2048h, 2048h+2048) and vocab columns
[8000q, 8000q+8000).  The cross-vocab-shard softmax sum is one AllReduce
over 4 ranks per tapered group of row-blocks (GS), in two independent
replica groups ([[0,1,2,3],[4,5,6,7]]).  The copy branch stays
batch-sharded 8 ways (4 batches/core).  A tiny warmup NEFF with one
AllReduce runs first (collective channel start latency ~60-75us).

v2 changes vs the bf16 baseline (374us):
  - Logits matmul in fp8e4 DoubleRow (2 K-planes per PE pass): W is
    pre-scaled by 32 on the host so its ~N(0,0.02) entries stay in
    fp8e4 normal range; the exp activation applies scale=1/32.
    Halves PE time (272us busy -> ~140us).
  - exp tiles stored fp8 (values in [e^-3, e^3], 3% rel err vs a 2e-2
    budget) - 2KB/partition/tile lets the pipeline run 48 tiles deep
    across the AllReduce latency.
  - One exp per 2048-col PSUM tile (PSUM = 2 x [128,2048] covering all
    8 banks; gates and the copy branch use slices of the same pool)
    instead of per 1024 - halves ACT instruction overhead.
  - Output og stored as fp8 of 8192*prob (host divides by 8192 on
    upcast) - halves the dominant og store traffic vs bf16.
  - The per-group sum reduce moved off the DVE FIFO (to gpsimd) and the
    AllReduce result fetch to the SP queue, so AR g+1's posting no
    longer chains behind AR g's scale drain on DVE.  In the baseline
    that false dependency serialized the 6 ARs at ~40us each across the
    whole back half of the kernel.
  - Gate logit matmuls in plain fp8 (N=1); e8 = 8192*exp(-x) comes
    straight out of the Exp activation via bias=ln(8192)-b_copy.
"""

import math
import os
import sys

for _p in ("/opt/trn_rl_repo", "/root/.axon_site/_ro/trn_rl_repo"):
    if os.path.isdir(_p) and _p not in sys.path:
        sys.path.insert(0, _p)

import numpy as np
import ml_dtypes

import concourse.bacc as bacc
import concourse.tile as tile
from concourse import mybir
from concourse.bass_utils import run_bass_kernel_spmd

# ---------------------------------------------------------------------------
# Problem dimensions (hardcoded per spec)
# ---------------------------------------------------------------------------
B, T, S, V, CV, D = 32, 128, 400, 32000, 600, 512
PAD = 1
NCORES = 8
NQ = 4                    # vocab shards
NH = 2                    # row halves
R = B * T                 # 4096 rows
VS = V // NQ              # 8000 vocab columns per core
RH = R // NH              # 2048 rows per core
RB = 128                  # rows per block (= one batch: T == 128)
NBL = RH // RB            # 16 row blocks per core
GS = [3, 3, 3, 3, 3, 1]   # all-reduce groups; tiny last group = short tail
NG = len(GS)
GOFF = [sum(GS[:i]) for i in range(NG)]
GRPOF = []                # block -> (group, index-in-group)
for _g, _n in enumerate(GS):
    for _j in range(_n):
        GRPOF.append((_g, _j))
LB = B // NCORES          # 4 local batches per core (copy branch)
KC = D // 128             # 4 contraction chunks of 128
# vocab chunking within a block (PSUM: [128,2048]f32 = 4 banks)
VCH = [2048, 2048, 2048, 1856]   # = 8000
VOFF = [0, 2048, 4096, 6144]
NVC = len(VCH)
# s-dim chunks for the copy branch: 400 = 128+128+128+16
SCH = [128, 128, 128, 16]
SOFF = [0, 128, 256, 384]

WSCALE = 32.0             # host multiplies W/w_copy by this before fp8 cast
OUT_SCALE = 8192.0        # og holds OUT_SCALE * prob in fp8

F32 = mybir.dt.float32
F16 = mybir.dt.float16
BF16 = mybir.dt.bfloat16
FP8 = mybir.dt.float8e4
NP_FP8 = ml_dtypes.float8_e4m3

EXP_BUFS = 48   # in-flight exp tiles ([128,2048] fp8)
DR = mybir.MatmulPerfMode.DoubleRow


def _mm_splits(n):
    """Split a free-dim span into <=512 pieces aligned to 512 (PSUM banks)."""
    out = []
    off = 0
    while off < n:
        w = min(512, n - off)
        out.append((off, w))
        off += w
    return out


def build_program(with_bias: bool, b_copy: float, pad_corr: float):
    nc = bacc.Bacc()

    hT8 = nc.dram_tensor("hT8", [128, KC, RH], FP8, kind="ExternalInput")
    wT8 = nc.dram_tensor("wT8", [128, KC, VS], FP8, kind="ExternalInput")
    wc8 = nc.dram_tensor("wc8", [128, KC], FP8, kind="ExternalInput")
    hTlh = nc.dram_tensor("hTlh", [D, LB * RB], BF16, kind="ExternalInput")
    hTll = nc.dram_tensor("hTll", [D, LB * RB], BF16, kind="ExternalInput")
    wch = nc.dram_tensor("wch", [D, 1], BF16, kind="ExternalInput")
    wcl = nc.dram_tensor("wcl", [D, 1], BF16, kind="ExternalInput")
    attnT = nc.dram_tensor("attnT", [S, LB * RB], F16, kind="ExternalInput")
    smap = nc.dram_tensor("smap", [LB, S, CV], F16, kind="ExternalInput")
    if with_bias:
        ebb = nc.dram_tensor("ebb", [128, VS], F32, kind="ExternalInput")

    og = nc.dram_tensor("og", [RH, VS], FP8, kind="ExternalOutput")
    oc = nc.dram_tensor("oc", [LB * RB, CV], F32, kind="ExternalOutput")

    with tile.TileContext(nc) as tc:
        with (
            tc.tile_pool(name="const", bufs=1) as const,
            tc.tile_pool(name="pm", bufs=2, space="PSUM") as pm,
            tc.tile_pool(name="expp", bufs=EXP_BUFS) as expp,
            tc.tile_pool(name="outp", bufs=2) as outp,
            tc.tile_pool(name="ocp", bufs=2) as ocp,
            tc.tile_pool(name="smapp", bufs=4) as smapp,
            tc.tile_pool(name="small", bufs=10) as small,
            tc.tile_pool(name="gatep", bufs=NBL) as gatep,
            tc.tile_pool(name="dram", bufs=1, space="DRAM") as dram,
        ):
            # ---------------- start-alignment barrier ----------------
            # A dummy AllReduce posted before anything else: the mesh
            # processes collectives in order, so the real per-group ARs see
            # launch-skew-aligned peers (v2 measured 12-27us of peer wait on
            # EVERY group AR = the 8 NEFF launches are staggered).  Nobody
            # waits on its output; the wait happens on the CC core while the
            # weight DMAs stream.
            bar_i = dram.tile([128, 4], F32, tag="bar_i", name="bar_i")
            bar_o = dram.tile([128, 4], F32, tag="bar_o", name="bar_o")
            bar_s = small.tile([128, 4], F32, tag="bar_s", name="bar_s")
            nc.gpsimd.memset(bar_s[:], 0)
            nc.gpsimd.dma_start(bar_i[:], bar_s[:])
            nc.gpsimd.collective_compute(
                "AllReduce",
                mybir.AluOpType.add,
                replica_groups=[[0, 1, 2, 3], [4, 5, 6, 7]],
                ins=[bar_i.opt()],
                outs=[bar_o.opt()],
            )

            # ---------------- prologue: resident loads ----------------
            # fp8 gen-branch residents first: gates need hT8 + wc8 (gpsimd
            # ring), the 4MB W shard is split per (vocab chunk, k-plane)
            # with k0/k1/k2 on the SP/ACT/DVE rings and k3 on gpsimd, chunk-
            # major so block 0's chunks become ready in consumption order.
            hT8_t = const.tile([128, KC, RH], FP8, tag="hT8", name="hT8")
            nc.gpsimd.dma_start(hT8_t[:], hT8[:])
            wc8_t = const.tile([128, KC], FP8, tag="wc8", name="wc8")
            nc.gpsimd.dma_start(wc8_t[:], wc8[:])
            wT8_t = const.tile([128, KC, VS], FP8, tag="wT8", name="wT8")
            w_eng = [nc.sync, nc.sync, nc.scalar, nc.scalar]
            for c in range(NVC):
                vo, ve = VOFF[c], VOFF[c] + VCH[c]
                for k in range(KC):
                    w_eng[k].dma_start(
                        wT8_t[:, k:k + 1, vo:ve], wT8[:, k:k + 1, vo:ve]
                    )
            # copy-branch inputs: bf16 hi/lo gate + attn behind hT8 on the
            # gpsimd ring, smap on the ACT ring behind its wT8 share; the
            # copy branch is emitted after group 0 to match arrival order
            hTl_t = []
            wc_t = []
            attnT_t = []
            ebb_t = []
            for k in range(KC):
                th = const.tile([128, 1], BF16, tag=f"wch{k}", name=f"wch{k}")
                nc.gpsimd.dma_start(th[:], wch[k * 128:(k + 1) * 128, :])
                tl = const.tile([128, 1], BF16, tag=f"wcl{k}", name=f"wcl{k}")
                nc.gpsimd.dma_start(tl[:], wcl[k * 128:(k + 1) * 128, :])
                wc_t.append((th, tl))
            for k in range(KC):
                th = const.tile([128, LB * RB], BF16, tag=f"hTlh{k}", name=f"hTlh{k}")
                nc.gpsimd.dma_start(th[:], hTlh[k * 128:(k + 1) * 128, :])
                tl = const.tile([128, LB * RB], BF16, tag=f"hTll{k}", name=f"hTll{k}")
                nc.gpsimd.dma_start(tl[:], hTll[k * 128:(k + 1) * 128, :])
                hTl_t.append((th, tl))
            for k in range(4):
                sk = SCH[k]
                t = const.tile([128, LB * RB], F16, tag=f"attnT{k}", name=f"attnT{k}")
                nc.gpsimd.dma_start(t[:sk, :], attnT[SOFF[k]:SOFF[k] + sk, :])
                attnT_t.append(t)
            if with_bias:
                for i in range(NVC):
                    t = const.tile([128, VCH[i]], F32, tag=f"ebb{i}", name=f"ebb{i}")
                    nc.sync.dma_start(t[:], ebb[:, VOFF[i]:VOFF[i] + VCH[i]])
                    ebb_t.append(t)

            # gate-exp bias constants as [128,1] tiles (only 0.0/1.0 floats
            # are pre-registered in the const-AP database)
            be8_t = const.tile([128, 1], F32, tag="be8", name="be8")
            nc.gpsimd.memset(be8_t[:], math.log(OUT_SCALE) - float(b_copy))
            bcl_t = const.tile([128, 1], F32, tag="bcl", name="bcl")
            nc.gpsimd.memset(bcl_t[:], -float(b_copy))

            # ---------------- main loop state ----------------
            exp_tiles = [[None] * NVC for _ in range(NBL)]
            e8_tiles = [None] * NBL   # 8192*exp(-gate logit) per block
            u_tiles = [None] * NBL    # 1 + e per block [128,1] f32
            cin_t = [None] * NG       # group AR inputs [128, 4*gn] (DRAM)
            cc_out = [None] * NG      # group all-reduced sums (DRAM)

            def compute_gate(jb):
                cb = slice(jb * RB, (jb + 1) * RB)
                # gate matmul (plain fp8, N=1) -> psum [128, 1]
                gps = pm.tile([128, 2048], F32, tag="pm", name="gps")
                for k in range(KC):
                    nc.tensor.matmul(
                        gps[:, 0:1], hT8_t[:, k:k + 1, cb], wc8_t[:, k:k + 1],
                        start=(k == 0), stop=(k == KC - 1),
                    )
                # e8 = OUT_SCALE * exp(-(x/WSCALE + b_copy)) via the Exp table
                e8 = gatep.tile([128, 1], F32, tag="e8", name="e8")
                nc.scalar.activation(
                    e8[:], gps[:, 0:1], mybir.ActivationFunctionType.Exp,
                    bias=be8_t[:],
                    scale=-1.0 / WSCALE,
                )
                # u = 1 + e = 1 + e8/OUT_SCALE
                u = gatep.tile([128, 1], F32, tag="u", name="u")
                nc.vector.tensor_scalar(
                    u[:], e8[:], 1.0 / OUT_SCALE, 1.0,
                    mybir.AluOpType.mult, mybir.AluOpType.add,
                )
                e8_tiles[jb] = e8
                u_tiles[jb] = u

            def compute_block(jb):
                cb = slice(jb * RB, (jb + 1) * RB)
                sp = small.tile([128, NVC], F32, tag="sp", name="sp")
                for i in range(NVC):
                    n = VCH[i]
                    ps = pm.tile([128, 2048], F32, tag="pm", name="pm")
                    for j in range(2):  # kk pairs (DoubleRow: 2 K-planes)
                        for (o, w) in _mm_splits(n):
                            nc.tensor.matmul(
                                ps[:, o:o + w],
                                hT8_t[:, 2 * j:2 * j + 2, cb],
                                wT8_t[:, 2 * j:2 * j + 2,
                                      VOFF[i] + o:VOFF[i] + o + w],
                                start=(j == 0), stop=(j == 1),
                                perf_mode=DR,
                            )
                    ex = expp.tile([128, 2048], FP8, tag="exp", name="exp")
                    if not with_bias:
                        nc.scalar.activation(
                            ex[:, :n], ps[:, :n],
                            mybir.ActivationFunctionType.Exp,
                            scale=1.0 / WSCALE,
                            accum_out=sp[:, i:i + 1],
                        )
                    else:
                        nc.scalar.activation(
                            ex[:, :n], ps[:, :n],
                            mybir.ActivationFunctionType.Exp,
                            scale=1.0 / WSCALE,
                        )
                        nc.vector.tensor_tensor(
                            ex[:, :n], ex[:, :n], ebb_t[i][:, :n],
                            mybir.AluOpType.mult,
                        )
                        nc.vector.reduce_sum(
                            sp[:, i:i + 1], ex[:, :n],
                            axis=mybir.AxisListType.X,
                        )
                    exp_tiles[jb][i] = ex
                g, j = GRPOF[jb]
                # off the DVE FIFO: ship the raw per-chunk sums; the AR sums
                # elementwise and the 4-way fold moves to the scale side.
                # AR posting never chains behind the previous group's scales.
                nc.gpsimd.dma_start(cin_t[g][:, NVC * j:NVC * (j + 1)], sp[:])

            def scale_block(jb, sgl):
                g, j = GRPOF[jb]
                # m8 = OUT_SCALE*(1-gate)/S = e8 / ((1+e) * (S_ar - pad_corr))
                ssum = small.tile([128, 1], F32, tag="ssum", name="ssum")
                nc.vector.reduce_sum(
                    ssum[:], sgl[:, NVC * j:NVC * (j + 1)],
                    axis=mybir.AxisListType.X,
                )
                corr = small.tile([128, 1], F32, tag="corr", name="corr")
                nc.vector.tensor_scalar_add(corr[:], ssum[:], -pad_corr)
                v = small.tile([128, 1], F32, tag="v", name="v")
                nc.vector.tensor_scalar(
                    v[:], corr[:], u_tiles[jb][:], None, mybir.AluOpType.mult
                )
                rec = small.tile([128, 1], F32, tag="rec", name="rec")
                nc.vector.reciprocal(rec[:], v[:])
                m8 = small.tile([128, 1], F32, tag="m8", name="m8")
                nc.vector.tensor_scalar(
                    m8[:], rec[:], e8_tiles[jb][:], None, mybir.AluOpType.mult
                )
                ot = outp.tile([128, VS], FP8, tag="ot", name="ot")
                for i in range(NVC):
                    n = VCH[i]
                    nc.vector.tensor_scalar(
                        ot[:, VOFF[i]:VOFF[i] + n],
                        exp_tiles[jb][i][:, :n], m8[:], None,
                        mybir.AluOpType.mult,
                    )
                    exp_tiles[jb][i] = None
                nc.sync.dma_start(og[jb * RB:(jb + 1) * RB, :], ot[:])

            # ---------------- copy branch (batch-sharded) ----------------
            def emit_copy_branch():
                # emitted first: no dependence on collectives or the big weights
                for l in range(LB):
                    tb = slice(l * RB, (l + 1) * RB)
                    # local gate: fp32-grade dot via bf16 hi/lo split
                    gps = pm.tile([128, 2048], F32, tag="pm", name="cgps")
                    nmm = 3 * KC
                    imm = 0
                    for k in range(KC):
                        for (a, b_) in ((0, 0), (0, 1), (1, 0)):
                            nc.tensor.matmul(
                                gps[:, 0:1], hTl_t[k][a][:, tb], wc_t[k][b_][:],
                                start=(imm == 0), stop=(imm == nmm - 1),
                            )
                            imm += 1
                    el = gatep.tile([128, 1], F32, tag="el", name="el")
                    nc.scalar.activation(
                        el[:], gps[:, 0:1], mybir.ActivationFunctionType.Exp,
                        bias=bcl_t[:], scale=-1.0,
                    )
                    ul = gatep.tile([128, 1], F32, tag="ul", name="ul")
                    nc.vector.tensor_scalar_add(ul[:], el[:], 1.0)
                    gl = gatep.tile([128, 1], F32, tag="gl", name="gl")
                    nc.vector.reciprocal(gl[:], ul[:])
                    cps = pm.tile([128, 2048], F32, tag="pm", name="cps")
                    for k in range(4):
                        sk = SCH[k]
                        sm = smapp.tile([128, CV], F16, tag="sm", name="sm")
                        nc.scalar.dma_start(
                            sm[:sk, :], smap[l, SOFF[k]:SOFF[k] + sk, :]
                        )
                        for (o, w) in _mm_splits(CV):
                            nc.tensor.matmul(
                                cps[:, o:o + w],
                                attnT_t[k][:sk, tb],
                                sm[:sk, o:o + w],
                                start=(k == 0), stop=(k == 3),
                            )
                    oct_ = ocp.tile([128, CV], F32, tag="oct", name="oct")
                    nc.vector.tensor_scalar(
                        oct_[:], cps[:, :CV], gl[:], None, mybir.AluOpType.mult
                    )
                    nc.sync.dma_start(oc[tb, :], oct_[:])

            def emit_group(g):
                gn = GS[g]
                cin_t[g] = dram.tile(
                    [128, NVC * gn], F32, tag=f"cin{g}", name=f"cin{g}"
                )
                for j in range(gn):
                    compute_block(GOFF[g] + j)
                # all-reduce this group's local sums across the 4 vocab shards
                cout = dram.tile(
                    [128, NVC * gn], F32, tag=f"cout{g}", name=f"cout{g}"
                )
                nc.gpsimd.collective_compute(
                    "AllReduce",
                    mybir.AluOpType.add,
                    replica_groups=[[0, 1, 2, 3], [4, 5, 6, 7]],
                    ins=[cin_t[g].opt()],
                    outs=[cout.opt()],
                )
                cc_out[g] = cout
                # result fetch on the SP queue (not gpsimd: later ARs must
                # keep posting; not DVE: the trigger's wait would stall it)
                sgl = small.tile([128, NVC * gn], F32, tag="sgl", name="sgl")
                nc.sync.dma_start(sgl[:], cout[:])
                for j in range(gn):
                    scale_block(GOFF[g] + j, sgl)

            # all 16 gates up-front: they only need hT8 + wc8 (small, fast
            # loads), so they fill the PE while the 4 MB wT8 shard streams
            # in.  The copy branch goes after group 0 — its inputs sit
            # behind hT8 on the gpsimd ring, so emitting it first would
            # stall the PE FIFO on DMAs while main-loop work is ready.
            for jb in range(NBL):
                compute_gate(jb)
            emit_group(0)
            emit_copy_branch()
            for g in range(1, NG):
                emit_group(g)

    nc.finalize()
    return nc


_warmed_up = False


def _warmup_collectives():
    """Run a minimal NEFF with one AllReduce so the collective channel
    (ncfw firmware / TOPSP) is warm before the main kernel executes —
    the first collective after boot costs ~60-75us of start latency."""
    global _warmed_up
    if _warmed_up:
        return
    nc = bacc.Bacc()
    x = nc.dram_tensor("x", [128, 4], F32, kind="ExternalInput")
    y = nc.dram_tensor("y", [128, 4], F32, kind="ExternalOutput")
    with tile.TileContext(nc) as tc:
        with (
            tc.tile_pool(name="sb", bufs=2) as sb,
            tc.tile_pool(name="dr", bufs=2, space="DRAM") as dr,
        ):
            t = sb.tile([128, 4], F32, tag="t", name="t")
            nc.sync.dma_start(t[:], x[:])
            bi = dr.tile([128, 4], F32, tag="bi", name="bi")
            bo = dr.tile([128, 4], F32, tag="bo", name="bo")
            nc.sync.dma_start(bi[:], t[:])
            nc.gpsimd.collective_compute(
                "AllReduce",
                mybir.AluOpType.add,
                replica_groups=[[0, 1, 2, 3], [4, 5, 6, 7]],
                ins=[bi.opt()],
                outs=[bo.opt()],
            )
            t2 = sb.tile([128, 4], F32, tag="t2", name="t2")
            nc.sync.dma_start(t2[:], bo[:])
            nc.sync.dma_start(y[:], t2[:])
    nc.finalize()
    z = np.zeros((128, 4), np.float32)
    run_bass_kernel_spmd(nc, [{"x": z}] * NCORES, core_ids=list(range(NCORES)))
    _warmed_up = True


def kernel(hidden, copy_attn, src_map, W, b, w_copy, b_copy, _trace=False):
    hidden = np.asarray(hidden, np.float32)
    copy_attn = np.asarray(copy_attn, np.float32)
    src_map = np.asarray(src_map, np.float32)
    W = np.asarray(W, np.float32)
    b = np.asarray(b, np.float32)
    w_copy = np.asarray(w_copy, np.float32)
    b_copy_f = float(np.asarray(b_copy))

    with_bias = bool(np.any(b != 0.0))
    pad_corr = float(np.exp(b[PAD])) if with_bias else 1.0

    # host-side shard prep (layout only; W[PAD,:] is dead data in the ref)
    Wz = W.copy()
    Wz[PAD, :] = 0.0
    # [128, KC, V]: fp8 of 32*W.T, partition-major for a single-tile resident
    W8 = np.ascontiguousarray(
        (Wz.T * WSCALE).reshape(KC, 128, V).transpose(1, 0, 2)
    ).astype(NP_FP8)
    hT_f = np.ascontiguousarray(hidden.T)                            # [D, R] f32
    h8 = np.ascontiguousarray(
        hT_f.reshape(KC, 128, R).transpose(1, 0, 2)
    ).astype(NP_FP8)                                                 # [128, KC, R]
    wc8_full = np.ascontiguousarray(
        (w_copy * WSCALE).reshape(KC, 128).T
    ).astype(NP_FP8)                                                 # [128, KC]
    hT_b = hT_f.astype(ml_dtypes.bfloat16)
    hT_lo = (hT_f - hT_b.astype(np.float32)).astype(ml_dtypes.bfloat16)
    wc32 = w_copy.reshape(D, 1).astype(np.float32)
    wc_hi = wc32.astype(ml_dtypes.bfloat16)
    wc_lo = (wc32 - wc_hi.astype(np.float32)).astype(ml_dtypes.bfloat16)
    attnT_full = np.ascontiguousarray(copy_attn.T).astype(np.float16)  # [S, R]
    smap16 = src_map.astype(np.float16)                              # [B,S,CV]

    _warmup_collectives()
    nc = build_program(with_bias, b_copy_f, pad_corr)

    in_maps = []
    for c in range(NCORES):
        h, q = divmod(c, NQ)
        rows = slice(h * RH, (h + 1) * RH)
        crows = slice(c * LB * RB, (c + 1) * LB * RB)
        m = {
            "hT8": np.ascontiguousarray(h8[:, :, rows]),
            "wT8": np.ascontiguousarray(W8[:, :, q * VS:(q + 1) * VS]),
            "wc8": wc8_full,
            "hTlh": np.ascontiguousarray(hT_b[:, crows]),
            "hTll": np.ascontiguousarray(hT_lo[:, crows]),
            "wch": wc_hi,
            "wcl": wc_lo,
            "attnT": np.ascontiguousarray(attnT_full[:, crows]),
            "smap": np.ascontiguousarray(smap16[c * LB:(c + 1) * LB]),
        }
        if with_bias:
            eb = np.exp(b[q * VS:(q + 1) * VS].astype(np.float64)).astype(
                np.float32
            )
            m["ebb"] = np.ascontiguousarray(
                np.broadcast_to(eb[None, :], (128, VS))
            )
        in_maps.append(m)

    trace_cores = None
    if os.environ.get("TRACE_ALL_CORES"):
        trace_cores = list(range(NCORES))
    res = run_bass_kernel_spmd(
        nc, in_maps, core_ids=list(range(NCORES)), trace=_trace,
        trace_cores=trace_cores,
    )

    out = np.empty((R, V + CV), np.float32)
    inv = np.float32(1.0 / OUT_SCALE)
    for c in range(NCORES):
        h, q = divmod(c, NQ)
        out[h * RH:(h + 1) * RH, q * VS:(q + 1) * VS] = (
            res.results[c]["og"].astype(np.float32) * inv
        )
        out[c * LB * RB:(c + 1) * LB * RB, V:] = res.results[c]["oc"]
    out[:, PAD] = 0.0

    if _trace:
        kernel.last_results = res
    return out


kernel.last_results = None
